# revision 19
# baseline (speedup 1.0000x reference)
"""Mixture-of-Experts (N=16384, D=1024, E=8, top-2) on 8 Trainium2 NeuronCores.

Expert-parallel with load balancing: the host computes the (tiny) router
decisions, then packs (expert, token-chunk) work units into 8 equal-capacity
cores. Each core holds up to M_SLOTS resident transposed expert weights in
SBUF; the tile->slot mapping is uniform across cores (SPMD) while the slot
contents (which expert, which token rows) vary per core via input data.

Per tile of 128 assignments: indirect-DMA gather of token rows from x,
PE transpose (float32r single-pass), 16 accumulating float32r matmuls
against the slot's W_e.T, bias add + renormalized top-2 gate on DVE, store.
The host scatter-adds each token's two contributions back to token order.
"""

import numpy as np

import concourse.bass as bass
import concourse.mybir as mybir
from concourse.tile import TileContext
from concourse.masks import make_identity
from concourse.bass_utils import run_bass_kernel_spmd

N, D, E, TOP_K = 16384, 1024, 8, 2
P = 128
F32 = mybir.dt.float32
F32R = mybir.dt.float32r
I32 = mybir.dt.int32


def _split_multiwaits(nc, max_waits=1):
    """This walrus build accepts at most one sync wait per engine instruction.

    Hoist excess waits onto standalone EventSemaphore carriers (the same
    instruction wait_ge emits) immediately before the instruction.
    """
    for fn in nc.m.functions:
        for bb in fn.blocks:
            new = []
            for inst in bb.instructions:
                si = inst.sync_info
                if (
                    si is not None
                    and si.on_wait
                    and len(si.on_wait) > max_waits
                    and not isinstance(inst, mybir.InstEventSemaphore)
                ):
                    waits = list(si.on_wait)
                    for w in waits[:-max_waits]:
                        c = mybir.InstEventSemaphore(
                            name=f"{inst.name}-hw{w.id}", ins=[], outs=[]
                        )
                        c.engine = inst.engine
                        c.sync_info = mybir.SyncInfo(on_wait=[w], on_update=[])
                        new.append(c)
                    si.on_wait = waits[-max_waits:]
                new.append(inst)
            bb.instructions = new


def _route_host(x, context_vector, router_w, router_b, context_weight):
    logits = x @ router_w.T + router_b + context_weight[0] * context_vector
    m = logits.max(axis=1, keepdims=True)
    ex = np.exp(logits - m)
    probs = ex / ex.sum(axis=1, keepdims=True)
    # descending stable sort == jax.lax.top_k tie-breaking (lower index first)
    top2 = np.argsort(-probs, axis=1, kind="stable")[:, :TOP_K]
    p12 = np.take_along_axis(probs, top2, axis=1)
    gates = (p12 / p12.sum(axis=1, keepdims=True)).astype(np.float32)

    counts = np.bincount(top2.ravel(), minlength=E)
    avg_prob = probs.mean(axis=0)
    load_loss = np.float32(np.sum(avg_prob * (counts / N)))
    return top2, gates, counts, load_loss


def _greedy_pack(tiles, slot_sizes):
    """Place each expert's tile count into per-core slots (8 cores x slots of
    the given sizes). Returns None if infeasible, else placement:
    list per core of list per slot of (expert, n_tiles) or None."""
    slots = []
    for c in range(E):
        for s, sz in enumerate(slot_sizes):
            slots.append((sz, c, s))
    slots.sort(key=lambda t: -t[0])
    placement = [[None] * len(slot_sizes) for _ in range(E)]
    order = sorted(range(E), key=lambda e: -tiles[e])
    free = [True] * len(slots)
    for e in order:
        rem = tiles[e]
        while rem > 0:
            # smallest free slot that fits rem entirely, else the largest free
            best = None
            for j, (sz, c, s) in enumerate(slots):
                if not free[j]:
                    continue
                if best is None:
                    best = j
                elif sz >= rem and (
                    slots[best][0] < rem or sz < slots[best][0]
                ):
                    best = j
            if best is None:
                return None
            sz, c, s = slots[best]
            take = min(rem, sz)
            placement[c][s] = (e, take)
            free[best] = False
            rem -= take
    return placement


def _solve_balance(counts):
    """Choose uniform slot sizes and a packing minimizing per-core tiles."""
    tiles = [-(-int(c) // P) for c in counts]
    cap_ub = max(tiles)
    best = (cap_ub, [cap_ub], _greedy_pack(tiles, [cap_ub]))

    def comps(total_max, m, prev):
        if m == 1:
            for t in range(min(total_max, prev), 0, -1):
                yield [t]
            return
        for t in range(min(total_max, prev), m - 2, -1):
            for rest in comps(total_max - t, m - 1, t):
                yield [t] + rest

    tried = 0
    for m in range(2, 5):
        for sizes in comps(cap_ub, m, cap_ub):
            tried += 1
            if tried > 200_000:  # bound solver time for pathological skews
                return best
            cap = sum(sizes)
            if cap >= best[0] or cap * E < sum(tiles):
                continue
            pl = _greedy_pack(tiles, sizes)
            if pl is not None:
                best = (cap, sizes, pl)
    return best


def _build(slot_bounds):
    """slot_bounds: (start_tile, end_tile) per slot, uniform across cores."""
    m_slots = len(slot_bounds)
    n_tiles = slot_bounds[-1][1]
    C = n_tiles * P
    KT = D // P  # 8 K-subtiles
    NC_ = D // 512  # 2 output chunks

    nc = bass.Bass()
    x = nc.declare_dram_parameter("x", [N, D], F32, isOutput=False)
    wt = nc.declare_dram_parameter("wt", [m_slots, D, D], F32, isOutput=False)
    bias = nc.declare_dram_parameter("bias", [m_slots, D], F32, isOutput=False)
    idx = nc.declare_dram_parameter("idx", [P, n_tiles], I32, isOutput=False)
    gate = nc.declare_dram_parameter("gate", [P, n_tiles], F32, isOutput=False)
    out = nc.declare_dram_parameter("out", [C, D], F32, isOutput=True)

    def slot_of(i):
        for s, (a, b) in enumerate(slot_bounds):
            if a <= i < b:
                return s
        raise AssertionError(i)

    with TileContext(nc) as tc:
        with (
            tc.tile_pool(name="const", bufs=1) as const,
            tc.tile_pool(name="xg", bufs=6) as xgp,
            tc.tile_pool(name="xt", bufs=3) as xtp,
            tc.tile_pool(name="outp", bufs=3) as outp,
            tc.tile_pool(name="pst", bufs=5, space="PSUM") as pst,
            tc.tile_pool(name="pso", bufs=3, space="PSUM") as pso,
        ):
            # tile indices resident up front (tiny DMA), so gathers never
            # queue behind the weight stream
            idx_sb = const.tile([P, n_tiles], I32)
            nc.sync.dma_start(idx_sb[:], idx[:, :])

            def gather_tile(i):
                xg = xgp.tile([P, D], F32R, name="xg")
                nc.gpsimd.indirect_dma_start(
                    out=xg[:],
                    out_offset=None,
                    in_=x[:, :].bitcast(F32R),
                    in_offset=bass.IndirectOffsetOnAxis(
                        ap=idx_sb[:, i : i + 1], axis=0
                    ),
                )
                return xg

            ident32 = const.tile([P, P], F32)
            make_identity(nc, ident32)
            ident = const.tile([P, P], F32R)
            nc.vector.tensor_copy(ident[:], ident32[:])

            # prefetch the first gathers so the PE starts transposing
            # immediately while slot-0 weights stream in
            PF = min(4, n_tiles)
            pref = [gather_tile(i) for i in range(PF)]

            gate_sb = const.tile([P, n_tiles], F32)
            nc.sync.dma_start(gate_sb[:], gate[:, :])

            # weights/bias are loaded per slot just-in-time: slot 0 up front,
            # each later slot right before the first tile that uses it, so the
            # 4MB/slot streams overlap the previous slot's compute.
            wts = [None] * m_slots
            bias_bb = [None] * m_slots

            def load_slot(s):
                wt_v = wt[s, :, :].rearrange("(k p) o -> p k o", p=P).bitcast(F32R)
                row = []
                for k in range(KT):
                    wk = const.tile([P, D], F32R, name=f"wts{s}_{k}")
                    nc.sync.dma_start(out=wk[:], in_=wt_v[:, k, :])
                    row.append(wk)
                wts[s] = row
                bb = const.tile([P, D], F32, name=f"bias{s}")
                nc.sync.dma_start(
                    out=bb[:], in_=bias[s : s + 1, :].to_broadcast([P, D])
                )
                bias_bb[s] = bb

            load_slot(0)
            slot_triggers = {}
            for s2 in range(1, m_slots):
                slot_triggers.setdefault(
                    max(slot_bounds[s2][0] - 2, 0), []
                ).append(s2)

            for i in range(n_tiles):
                s = slot_of(i)
                # issue upcoming slots' weight streams a couple tiles early
                for s2 in slot_triggers.get(i, []):
                    load_slot(s2)
                xg = pref[i] if i < PF else gather_tile(i)

                # transpose to xt[p, k, t] (d = k*128+p on partitions)
                xt = xtp.tile([P, KT, P], F32R)
                for k in range(KT):
                    ps = pst.tile([P, P], F32, space="PSUM", name="pst")
                    nc.tensor.transpose(
                        ps[:].bitcast(F32R), xg[:, k * P : (k + 1) * P], ident[:]
                    )
                    # split copybacks between ACT and DVE to halve that path
                    if k % 2 == 0:
                        nc.scalar.copy(xt[:, k, :], ps[:])
                    else:
                        nc.vector.tensor_copy(xt[:, k, :], ps[:])

                outt = outp.tile([P, D], F32)
                for n in range(NC_):
                    po = pso.tile([P, 512], F32, space="PSUM", name="po")
                    for k in range(KT):
                        nc.tensor.matmul(
                            po[:],
                            xt[:, k, :],
                            wts[s][k][:, 512 * n : 512 * (n + 1)],
                            start=(k == 0),
                            stop=(k == KT - 1),
                        )
                    # out = psum + bias (PSUM -> SBUF)
                    nc.vector.tensor_add(
                        outt[:, 512 * n : 512 * (n + 1)],
                        po[:],
                        bias_bb[s][:, 512 * n : 512 * (n + 1)],
                    )
                # apply renormalized top-2 gate (zero on padding rows)
                nc.vector.tensor_scalar_mul(outt[:], outt[:], gate_sb[:, i : i + 1])
                nc.sync.dma_start(out[P * i : P * (i + 1), :], outt[:])

    _split_multiwaits(nc)
    return nc


def kernel(**inputs):
    x = np.ascontiguousarray(np.asarray(inputs["x"], dtype=np.float32))
    cv = np.asarray(inputs["context_vector"], dtype=np.float32)
    rw = np.asarray(inputs["router_w"], dtype=np.float32)
    rb = np.asarray(inputs["router_b"], dtype=np.float32)
    ew = np.asarray(inputs["expert_w"], dtype=np.float32)
    eb = np.asarray(inputs["expert_b"], dtype=np.float32)
    cw = np.asarray(inputs["context_weight"], dtype=np.float32)

    top2, gates, counts, load_loss = _route_host(x, cv, rw, rb, cw)

    tok_lists, gate_lists = [], []
    for e in range(E):
        sel = np.nonzero(top2 == e)
        tok_lists.append(sel[0].astype(np.int32))
        gate_lists.append(gates[sel[0], sel[1]].astype(np.float32))

    cap, slot_sizes, placement = _solve_balance(counts)
    m_slots = len(slot_sizes)
    bounds = []
    a = 0
    for sz in slot_sizes:
        bounds.append((a, a + sz))
        a += sz
    C = cap * P

    nc = _build(bounds)

    ewt = np.ascontiguousarray(np.transpose(ew, (0, 2, 1)))  # [E, D(in), D(out)]
    used = [0] * E  # per-expert row cursor (in tiles)
    in_maps = []
    core_layout = []  # per core: list of (expert, row0, nrows, row_offset)
    for c in range(E):
        idx_c = np.zeros((C,), np.int32)
        gate_c = np.zeros((C,), np.float32)
        wt_c = np.zeros((m_slots, D, D), np.float32)
        bias_c = np.zeros((m_slots, D), np.float32)
        layout = []
        for s in range(m_slots):
            pl = placement[c][s]
            if pl is None:
                continue
            e, ntile = pl
            wt_c[s] = ewt[e]
            bias_c[s] = eb[e]
            r0 = used[e] * P
            nrows = min(ntile * P, int(counts[e]) - r0)
            used[e] += ntile
            off = bounds[s][0] * P
            idx_c[off : off + nrows] = tok_lists[e][r0 : r0 + nrows]
            gate_c[off : off + nrows] = gate_lists[e][r0 : r0 + nrows]
            layout.append((e, r0, nrows, off))
        core_layout.append(layout)
        in_maps.append(
            {
                "x": x,
                "wt": wt_c,
                "bias": bias_c,
                # device wants [P, n_tiles]: tile-major -> transposed
                "idx": np.ascontiguousarray(idx_c.reshape(-1, P).T),
                "gate": np.ascontiguousarray(gate_c.reshape(-1, P).T),
            }
        )

    res = run_bass_kernel_spmd(nc, in_maps, list(range(E)))
    globals()["_last_results"] = res

    out = np.zeros((N, D), np.float32)
    for c in range(E):
        r = res.results[c]["out"]
        for e, r0, nrows, off in core_layout[c]:
            out[tok_lists[e][r0 : r0 + nrows]] += r[off : off + nrows]
    return out, load_loss


# revision 20
# speedup vs baseline: 1.0232x; 1.0232x over previous
"""Mixture-of-Experts (N=16384, D=1024, E=8, top-2) on 8 Trainium2 NeuronCores.

Expert-parallel with load balancing: the host computes the (tiny) router
decisions, then packs (expert, token-chunk) work units into 8 equal-capacity
cores. Each core holds up to M_SLOTS resident transposed expert weights in
SBUF; the tile->slot mapping is uniform across cores (SPMD) while the slot
contents (which expert, which token rows) vary per core via input data.

Per tile of 128 assignments: indirect-DMA gather of token rows from x,
PE transpose (float32r single-pass), 16 accumulating float32r matmuls
against the slot's W_e.T, bias add + renormalized top-2 gate on DVE, store.
The host scatter-adds each token's two contributions back to token order.
"""

import numpy as np

import concourse.bass as bass
import concourse.mybir as mybir
from concourse.tile import TileContext
from concourse.masks import make_identity
from concourse.bass_utils import run_bass_kernel_spmd

N, D, E, TOP_K = 16384, 1024, 8, 2
P = 128
F32 = mybir.dt.float32
F32R = mybir.dt.float32r
I32 = mybir.dt.int32


def _split_multiwaits(nc, max_waits=1):
    """This walrus build accepts at most one sync wait per engine instruction.

    Hoist excess waits onto standalone EventSemaphore carriers (the same
    instruction wait_ge emits) immediately before the instruction.
    """
    for fn in nc.m.functions:
        for bb in fn.blocks:
            new = []
            for inst in bb.instructions:
                si = inst.sync_info
                if (
                    si is not None
                    and si.on_wait
                    and len(si.on_wait) > max_waits
                    and not isinstance(inst, mybir.InstEventSemaphore)
                ):
                    waits = list(si.on_wait)
                    for w in waits[:-max_waits]:
                        c = mybir.InstEventSemaphore(
                            name=f"{inst.name}-hw{w.id}", ins=[], outs=[]
                        )
                        c.engine = inst.engine
                        c.sync_info = mybir.SyncInfo(on_wait=[w], on_update=[])
                        new.append(c)
                    si.on_wait = waits[-max_waits:]
                new.append(inst)
            bb.instructions = new


def _route_host(x, context_vector, router_w, router_b, context_weight):
    logits = x @ router_w.T + router_b + context_weight[0] * context_vector
    m = logits.max(axis=1, keepdims=True)
    ex = np.exp(logits - m)
    probs = ex / ex.sum(axis=1, keepdims=True)
    # descending stable sort == jax.lax.top_k tie-breaking (lower index first)
    top2 = np.argsort(-probs, axis=1, kind="stable")[:, :TOP_K]
    p12 = np.take_along_axis(probs, top2, axis=1)
    gates = (p12 / p12.sum(axis=1, keepdims=True)).astype(np.float32)

    counts = np.bincount(top2.ravel(), minlength=E)
    avg_prob = probs.mean(axis=0)
    load_loss = np.float32(np.sum(avg_prob * (counts / N)))
    return top2, gates, counts, load_loss


def _greedy_pack(tiles, slot_sizes):
    """Place each expert's tile count into per-core slots (8 cores x slots of
    the given sizes). Returns None if infeasible, else placement:
    list per core of list per slot of (expert, n_tiles) or None."""
    slots = []
    for c in range(E):
        for s, sz in enumerate(slot_sizes):
            slots.append((sz, c, s))
    slots.sort(key=lambda t: -t[0])
    placement = [[None] * len(slot_sizes) for _ in range(E)]
    order = sorted(range(E), key=lambda e: -tiles[e])
    free = [True] * len(slots)
    for e in order:
        rem = tiles[e]
        while rem > 0:
            # smallest free slot that fits rem entirely, else the largest free
            best = None
            for j, (sz, c, s) in enumerate(slots):
                if not free[j]:
                    continue
                if best is None:
                    best = j
                elif sz >= rem and (
                    slots[best][0] < rem or sz < slots[best][0]
                ):
                    best = j
            if best is None:
                return None
            sz, c, s = slots[best]
            take = min(rem, sz)
            placement[c][s] = (e, take)
            free[best] = False
            rem -= take
    return placement


def _solve_balance(counts):
    """Choose uniform slot sizes and a packing minimizing per-core tiles."""
    tiles = [-(-int(c) // P) for c in counts]
    cap_ub = max(tiles)
    best = (cap_ub, [cap_ub], _greedy_pack(tiles, [cap_ub]))

    def comps(total_max, m, prev):
        if m == 1:
            for t in range(min(total_max, prev), 0, -1):
                yield [t]
            return
        for t in range(min(total_max, prev), m - 2, -1):
            for rest in comps(total_max - t, m - 1, t):
                yield [t] + rest

    tried = 0
    for m in range(2, 5):
        for sizes in comps(cap_ub, m, cap_ub):
            tried += 1
            if tried > 200_000:  # bound solver time for pathological skews
                return best
            cap = sum(sizes)
            if cap >= best[0] or cap * E < sum(tiles):
                continue
            pl = _greedy_pack(tiles, sizes)
            if pl is not None:
                best = (cap, sizes, pl)
    return best


def _build(slot_bounds):
    """slot_bounds: (start_tile, end_tile) per slot, uniform across cores."""
    m_slots = len(slot_bounds)
    n_tiles = slot_bounds[-1][1]
    C = n_tiles * P
    KT = D // P  # 8 K-subtiles
    NC_ = D // 512  # 2 output chunks

    nc = bass.Bass()
    x = nc.declare_dram_parameter("x", [N, D], F32, isOutput=False)
    wt = nc.declare_dram_parameter("wt", [m_slots, D, D], F32, isOutput=False)
    bias = nc.declare_dram_parameter("bias", [m_slots, D], F32, isOutput=False)
    idx = nc.declare_dram_parameter("idx", [P, n_tiles], I32, isOutput=False)
    gate = nc.declare_dram_parameter("gate", [P, n_tiles], F32, isOutput=False)
    out = nc.declare_dram_parameter("out", [C, D], F32, isOutput=True)

    def slot_of(i):
        for s, (a, b) in enumerate(slot_bounds):
            if a <= i < b:
                return s
        raise AssertionError(i)

    with TileContext(nc) as tc:
        with (
            tc.tile_pool(name="const", bufs=1) as const,
            tc.tile_pool(name="xg", bufs=6) as xgp,
            tc.tile_pool(name="xt", bufs=3) as xtp,
            tc.tile_pool(name="outp", bufs=3) as outp,
            tc.tile_pool(name="pst", bufs=4, space="PSUM") as pst,
            tc.tile_pool(name="pso", bufs=4, space="PSUM") as pso,
        ):
            # warm the SWDGE indirect-DMA path during kernel-entry init so
            # the first real gather doesn't pay first-use setup (~4.5us)
            warm_idx = const.tile([P, 1], I32)
            nc.gpsimd.memset(warm_idx[:], 0)
            warm_xg = const.tile([P, D], F32R)
            nc.gpsimd.indirect_dma_start(
                out=warm_xg[:],
                out_offset=None,
                in_=x[:, :].bitcast(F32R),
                in_offset=bass.IndirectOffsetOnAxis(ap=warm_idx[:, :1], axis=0),
            )

            # tile indices resident up front (tiny DMA), so gathers never
            # queue behind the weight stream
            idx_sb = const.tile([P, n_tiles], I32)
            nc.sync.dma_start(idx_sb[:], idx[:, :])

            def gather_tile(i):
                xg = xgp.tile([P, D], F32R, name="xg")
                nc.gpsimd.indirect_dma_start(
                    out=xg[:],
                    out_offset=None,
                    in_=x[:, :].bitcast(F32R),
                    in_offset=bass.IndirectOffsetOnAxis(
                        ap=idx_sb[:, i : i + 1], axis=0
                    ),
                )
                return xg

            ident32 = const.tile([P, P], F32)
            make_identity(nc, ident32)
            ident = const.tile([P, P], F32R)
            nc.vector.tensor_copy(ident[:], ident32[:])

            # prefetch the first gathers so the PE starts transposing
            # immediately while slot-0 weights stream in
            PF = min(4, n_tiles)
            pref = [gather_tile(i) for i in range(PF)]

            gate_sb = const.tile([P, n_tiles], F32)
            nc.sync.dma_start(gate_sb[:], gate[:, :])

            # weights/bias are loaded per slot just-in-time: slot 0 up front,
            # each later slot right before the first tile that uses it, so the
            # 4MB/slot streams overlap the previous slot's compute.
            wts = [None] * m_slots
            bias_bb = [None] * m_slots

            def load_slot(s):
                wt_v = wt[s, :, :].rearrange("(k p) o -> p k o", p=P).bitcast(F32R)
                row = []
                for k in range(KT):
                    wk = const.tile([P, D], F32R, name=f"wts{s}_{k}")
                    nc.sync.dma_start(out=wk[:], in_=wt_v[:, k, :])
                    row.append(wk)
                wts[s] = row
                bb = const.tile([P, D], F32, name=f"bias{s}")
                nc.sync.dma_start(
                    out=bb[:], in_=bias[s : s + 1, :].to_broadcast([P, D])
                )
                bias_bb[s] = bb

            load_slot(0)
            slot_triggers = {}
            for s2 in range(1, m_slots):
                slot_triggers.setdefault(
                    max(slot_bounds[s2][0] - 2, 0), []
                ).append(s2)

            for i in range(n_tiles):
                s = slot_of(i)
                # issue upcoming slots' weight streams a couple tiles early
                for s2 in slot_triggers.get(i, []):
                    load_slot(s2)
                xg = pref[i] if i < PF else gather_tile(i)

                # transpose to xt[p, k, t] (d = k*128+p on partitions)
                xt = xtp.tile([P, KT, P], F32R)
                for k in range(KT):
                    ps = pst.tile([P, P], F32, space="PSUM", name="pst")
                    nc.tensor.transpose(
                        ps[:].bitcast(F32R), xg[:, k * P : (k + 1) * P], ident[:]
                    )
                    # split copybacks between ACT and DVE to halve that path
                    if k % 2 == 0:
                        nc.scalar.copy(xt[:, k, :], ps[:])
                    else:
                        nc.vector.tensor_copy(xt[:, k, :], ps[:])

                outt = outp.tile([P, D], F32)
                for n in range(NC_):
                    po = pso.tile([P, 512], F32, space="PSUM", name="po")
                    for k in range(KT):
                        nc.tensor.matmul(
                            po[:],
                            xt[:, k, :],
                            wts[s][k][:, 512 * n : 512 * (n + 1)],
                            start=(k == 0),
                            stop=(k == KT - 1),
                        )
                    # out = psum + bias (PSUM -> SBUF)
                    nc.vector.tensor_add(
                        outt[:, 512 * n : 512 * (n + 1)],
                        po[:],
                        bias_bb[s][:, 512 * n : 512 * (n + 1)],
                    )
                # apply renormalized top-2 gate (zero on padding rows)
                nc.vector.tensor_scalar_mul(outt[:], outt[:], gate_sb[:, i : i + 1])
                nc.sync.dma_start(out[P * i : P * (i + 1), :], outt[:])

    _split_multiwaits(nc)
    return nc


def kernel(**inputs):
    x = np.ascontiguousarray(np.asarray(inputs["x"], dtype=np.float32))
    cv = np.asarray(inputs["context_vector"], dtype=np.float32)
    rw = np.asarray(inputs["router_w"], dtype=np.float32)
    rb = np.asarray(inputs["router_b"], dtype=np.float32)
    ew = np.asarray(inputs["expert_w"], dtype=np.float32)
    eb = np.asarray(inputs["expert_b"], dtype=np.float32)
    cw = np.asarray(inputs["context_weight"], dtype=np.float32)

    top2, gates, counts, load_loss = _route_host(x, cv, rw, rb, cw)

    tok_lists, gate_lists = [], []
    for e in range(E):
        sel = np.nonzero(top2 == e)
        tok_lists.append(sel[0].astype(np.int32))
        gate_lists.append(gates[sel[0], sel[1]].astype(np.float32))

    cap, slot_sizes, placement = _solve_balance(counts)
    m_slots = len(slot_sizes)
    bounds = []
    a = 0
    for sz in slot_sizes:
        bounds.append((a, a + sz))
        a += sz
    C = cap * P

    nc = _build(bounds)

    ewt = np.ascontiguousarray(np.transpose(ew, (0, 2, 1)))  # [E, D(in), D(out)]
    used = [0] * E  # per-expert row cursor (in tiles)
    in_maps = []
    core_layout = []  # per core: list of (expert, row0, nrows, row_offset)
    for c in range(E):
        idx_c = np.zeros((C,), np.int32)
        gate_c = np.zeros((C,), np.float32)
        wt_c = np.zeros((m_slots, D, D), np.float32)
        bias_c = np.zeros((m_slots, D), np.float32)
        layout = []
        for s in range(m_slots):
            pl = placement[c][s]
            if pl is None:
                continue
            e, ntile = pl
            wt_c[s] = ewt[e]
            bias_c[s] = eb[e]
            r0 = used[e] * P
            nrows = min(ntile * P, int(counts[e]) - r0)
            used[e] += ntile
            off = bounds[s][0] * P
            idx_c[off : off + nrows] = tok_lists[e][r0 : r0 + nrows]
            gate_c[off : off + nrows] = gate_lists[e][r0 : r0 + nrows]
            layout.append((e, r0, nrows, off))
        core_layout.append(layout)
        in_maps.append(
            {
                "x": x,
                "wt": wt_c,
                "bias": bias_c,
                # device wants [P, n_tiles]: tile-major -> transposed
                "idx": np.ascontiguousarray(idx_c.reshape(-1, P).T),
                "gate": np.ascontiguousarray(gate_c.reshape(-1, P).T),
            }
        )

    res = run_bass_kernel_spmd(nc, in_maps, list(range(E)))
    globals()["_last_results"] = res

    out = np.zeros((N, D), np.float32)
    for c in range(E):
        r = res.results[c]["out"]
        for e, r0, nrows, off in core_layout[c]:
            out[tok_lists[e][r0 : r0 + nrows]] += r[off : off + nrows]
    return out, load_loss
